# revision 65
# baseline (speedup 1.0000x reference)
"""Trainium2 Bass kernel for nn_AttentionLayer (B=4, S=4096, D=1024, fp32).

Sharding: 8 cores = 4 batches x 2 query-halves. Each core receives the
TRANSPOSED x rows of its own query half plus Wq (i-columns permuted), its
own j-half of Wk, Wv^T (i-rows permuted), bq and bv — all host-side layout
marshaling only (transpose / slice / permute / concatenate; values and
dtypes unchanged). Each core receives the FULL 4096-key xT (own query-half
keys first, then the partner half, in its own local order), so no x
collective exists; core pairs (same batch) exchange only M-halves and V
halves with local-output AllGathers. Each core computes single-head
attention for its query half and writes a [2048, 1024] fp32 slice; the
host gathers slices into [4, 4096, 1024].

Key algebraic restructure vs the direct formulation: softmax is invariant
to per-query score shifts, so with M := Wq^T Wk and u := bq^T Wk,

    S ~ Z x_all^T  (mod per-query shifts),   Z := x_own M + 1 (x) u

reproduces softmax(QK^T) EXACTLY (the bk and bq.bk cross terms are
per-query constants and cancel — bk is never needed on device). This
eliminates the K projection.

Precision/speed structure (v2): the two dominant matmuls (scores and
alpha.V — 78% of PE cycles in the bf16 formulation) run as fp8e4
DoubleRow matmuls (2 contraction chunks per instruction, 0.5 cycles/row:
4x the bf16 rate). Single-fp8 operands would blow the 2e-2 error budget
(measured ~2-5e-2 via numpy emulation), so every fp8 operand is a
compensated hi+lo pair (lo = fp8(x - fp8(x)), effective precision ~bf16)
and each matmul runs 3 of the 4 hi/lo cross terms (lo*lo dropped):

    scores: x8.z8 (+) x8.zlo (+) xlo.z8     [12 DR instrs / 512-q block]
    A.V:    a8.v8 (+) a8.vlo (+) alo.v8     [48 DR instrs / q-tile.chunk]

at 0.75x the bf16 cycle cost per matmul. The hi/lo pairs are packed on a
middle tile dim so one DoubleRow instruction computes both cross terms:
X8[:,ic,{x8,xlo},:] x Z8[:,ic,{zlo,z8},:] = x8.zlo + xlo.z8. All other
matmuls (M, Z and V projections) stay 16-bit, upgraded bf16 -> fp16
(same PE rate, 8x less rounding). The V hi+lo fp8 pair is the collective
payload (same bytes as one bf16 V). Z is unscaled (std ~1, ideal for
e4m3); the 1/32 score scale and a -1.5 shift (to keep exp under e4m3
max) fold into the ACT exp. alpha's ones-column denominator rides in the
v8 half (vlo ones-cols are 0), so den = sum(a8)+sum(alo) matches the
numerator's effective alpha exactly. Numpy emulation of this exact
rounding chain: rel err 1.7e-3 vs the 2e-2 gate (bf16 baseline: 6.4e-3);
measured on the hardware path: 2.16e-3.

Per-core PE work: M 33k + Z 131k + V 131k + scores 393k + AV 396k
~= 1086k cycles (bf16 baseline: 1348k). TimelineSim (the perf metric,
with collectives replaced by equivalent-volume local DMAs as in
test.py): 489494 ns vs the 594897 ns bf16 baseline. Phase B (scores+AV,
~331us) runs at ~100% PE occupancy; phase A (~154us) is bounded by the
26MB input-load stream on the single shared DMA device plus the
fp8-pair cast work on ACT/DVE, orchestrated via FIFO-emission-order
scheduling (see below).

Per-core program (SPMD, identical on all cores). DMA ring assignment:
sync(SP) = input loads + V stores + collective readbacks + out stores,
scalar(ACT) = M stores, gpsimd(Pool) = collective triggers. Engine
split for the drain/cast work: ACT = z16/z8/x8/v8 casts + exp + AV
drains; DVE = weight/x16 casts, lo-residual subs, v16 bias add, a8/alo.
  phase A: u = bq^T WkH chases the Wk DMAs; M-half = Wq^T WkH chases the
           Wq DMAs in 8 PSUM banks; M+u pair-AllGather (fp16 payload);
           x loads -> XT16 (own half) + X8 hi/lo (all keys); Z^T = M^T
           x_own^T -> Z8 hi/lo, SBUF-resident; V = x_own Wv^T + bv ->
           v8/vlo -> DRAM -> pair AllGather.
  phase B: V2 resident in SBUF (vlo,v8 pairs + ones-cols). Per 512-query
           block: 12 DR matmuls per key chunk -> exp on ACT -> a8/alo;
           out = (A.V)/den accumulated over 16 key-chunk-pairs x 3
           output chunks of 344; qi-outer AV ordering overlaps each
           query-tile's drain with the next tile's matmuls; the final
           tile runs denominator-chunk-first with split stores.
"""

import math
from contextlib import ExitStack

import numpy as np

import concourse.bass as bass
import concourse.tile as tile
from concourse import bacc, mybir

F32 = mybir.dt.float32
FP16 = mybir.dt.float16
F8 = mybir.dt.float8e4
P = 128

# Full-problem constants (hardcoded; harness provides matching inputs).
B, S_FULL, D = 4, 4096, 1024
N_CORES = 8
SQ = S_FULL // 2  # query rows per core

PAIR_GROUPS = [[0, 1], [2, 3], [4, 5], [6, 7]]

EBIAS = -1.5  # folded into exp; cancels in softmax, keeps a8 < e4m3 max


def build_module_cc(S, SQ_, D_, qblk=512, niter=1):
    """Build the per-core Bass program. S = key rows, SQ_ = query rows."""
    assert S == 2 * SQ_
    nc = bacc.Bacc(None, num_devices=N_CORES)
    DC = D_ // P          # 128-chunks of the model dim (8)

    xt_h = nc.dram_tensor("xT", [D_, S], F32, kind="ExternalInput")
    wq_h = nc.dram_tensor("Wq", [D_, D_], F32, kind="ExternalInput")
    wkh_h = nc.dram_tensor("WkH", [D_, D_ // 2], F32, kind="ExternalInput")
    wv_h = nc.dram_tensor("WvT", [D_, D_], F32, kind="ExternalInput")
    bqt_h = nc.dram_tensor("bqT", [P, D_ // P], F32, kind="ExternalInput")
    bv_h = nc.dram_tensor("bv", [D_], F32, kind="ExternalInput")
    out_h = nc.dram_tensor("out", [SQ_, D_], F32, kind="ExternalOutput")

    with tile.TileContext(nc) as tc, ExitStack() as ctx:
        consts = ctx.enter_context(tc.tile_pool(name="consts", bufs=1))
        dram = ctx.enter_context(tc.tile_pool(name="dram", bufs=1, space="DRAM"))

        # bq arrives host-striped as [P, DC] (element (p,c) = bq[c*128+p]);
        # loaded on the Pool ring so the SP load stream starts at t=0
        bqT = consts.tile([P, DC], F32)
        nc.gpsimd.dma_start(bqT, bqt_h[:, :])
        bqTb = consts.tile([P, DC], FP16)
        nc.vector.tensor_copy(bqTb, bqT)
        ebias_t = consts.tile([P, 1], F32)
        nc.vector.memset(ebias_t, EBIAS)
        # warm the ACT function tables (Exp for phase B, Identity for the
        # Z drains) at t~0 while ACT is idle, so no ACT_TABLE_LOAD lands
        # on the critical path later; the scrap result is never read
        warm = consts.tile([P, 1], F32)
        nc.scalar.activation(warm, ebias_t,
                             mybir.ActivationFunctionType.Exp)
        nc.scalar.activation(warm, ebias_t,
                             mybir.ActivationFunctionType.Identity,
                             bias=ebias_t)
        pid = nc.partition_id()

        for it in range(niter):
            _emit_iteration(
                nc, tc, dram, it, S, SQ_, D_, qblk,
                xt_h, wq_h, wkh_h, wv_h, bv_h, out_h,
                bqTb, ebias_t, pid,
            )

    nc.finalize()
    return nc


def _emit_iteration(nc, tc, dram, it, S, SQ_, D_, qblk,
                    xt_h, wq_h, wkh_h, wv_h, bv_h, out_h,
                    bqTb, ebias_t, pid):
    DC = D_ // P
    JH = D_ // 2          # j-half width (512)
    KC = S // P           # gathered key chunks (32)
    KCL = SQ_ // P        # local key chunks (16)
    NBLK = SQ_ // qblk    # query blocks (4)
    QT_PER_BLK = qblk // P
    scale = 1.0 / math.sqrt(D_)
    MW = DC * JH          # M-half payload cols (4096); + 4 for u
    DV = D_ + 8           # V cols + ones-column pad (1032 = 3*344)
    CH = 344

    with ExitStack() as itctx:
        x8p = itctx.enter_context(tc.tile_pool(name=f"x8p{it}", bufs=1))
        z8p = itctx.enter_context(tc.tile_pool(name=f"z8p{it}", bufs=1))
        up = itctx.enter_context(tc.tile_pool(name=f"up{it}", bufs=1))

        actx = ExitStack()
        ktp = actx.enter_context(tc.tile_pool(name=f"ktp{it}", bufs=1))
        mtp = actx.enter_context(tc.tile_pool(name=f"mtp{it}", bufs=1))
        wload = actx.enter_context(tc.tile_pool(name=f"wload{it}", bufs=3))
        xload = actx.enter_context(tc.tile_pool(name=f"xload{it}", bufs=3))

        consts_a = actx.enter_context(
            tc.tile_pool(name=f"consts_a{it}", bufs=1))
        # bv broadcast to all partitions, loaded at t=0 on the Pool ring
        # while the DMA device is otherwise idle
        bvb = consts_a.tile([P, D_], F32, name=f"bvb{it}")
        nc.gpsimd.dma_start(bvb, bv_h[None, :].to_broadcast([P, D_]))

        M_loc = dram.tile([P, MW + 4], FP16, name=f"M_loc{it}", tag=f"ML{it}")
        M_gath = dram.tile([2, P, MW + 4], FP16, name=f"M_gath{it}",
                           tag=f"MG{it}")
        V_loc = dram.tile([P, KCL, 2, D_], F8, name=f"V_loc{it}",
                          tag=f"VL{it}")
        V_gath = dram.tile([2, P, KCL, 2, D_], F8, name=f"V_gath{it}",
                           tag=f"VG{it}")

        # XT16: own-half keys only (Z and V projections); 16-bit.
        XT16 = ktp.tile([P, DC, SQ_], FP16, name=f"XT16{it}")
        # X8: hi/lo fp8 pair for ALL 4096 keys (scores lhsT). Pair dim:
        # index 0 = x8 = fp8(x), 1 = xlo = fp8(x - x8).
        X8 = x8p.tile([P, DC, 2, S], F8, name=f"X8{it}")
        # Z8: hi/lo fp8 pair, SBUF-resident. Pair dim: 0 = zlo, 1 = z8
        # (so X8[:,ic,:,:] x Z8[:,ic,:,:] = x8.zlo + xlo.z8 in one DR).
        Z8 = z8p.tile([P, DC, 2, SQ_], F8, name=f"Z8{it}")

        wqk = ExitStack()
        wqkp = wqk.enter_context(tc.tile_pool(name=f"wqkp{it}", bufs=1))
        wk_sb = wqkp.tile([P, DC, JH], FP16, name=f"wk_sb{it}")
        m_sb = mtp.tile([P, DC, D_], FP16, name=f"m_sb{it}")
        u32 = up.tile([P, DC], F32, name=f"u32{it}")

        # ---- phase A
        partner = (pid + 1) % 2

        # One shared phase-A PSUM pool: a single [P, 512] tag ring over
        # all 8 banks serves M, u, Z0, V and Z1 with no pool-swap stalls.
        psum_a = actx.enter_context(
            tc.tile_pool(name=f"psum_a{it}", bufs=8, space="PSUM"))

        # M-half = Wq^T Wk[:, own-j-half] chases interleaved Wk/Wq chunk
        # loads on the ACT DMA ring (each o-chunk of M needs only that
        # o-chunk of Wk and Wq, so the PE starts ~3us in); u = bq^T Wk
        # runs after M from the resident wk_sb into drained M banks.
        NXB = SQ_ // 512

        def x_block(xb, own, act_cast=False):
            c0 = xb * 512
            for icp in range(DC // 2):
                ic = icp * 2
                xf = xload.tile([P, 2, 512], F32, tag="ld", bufs=3,
                                name=f"xf{it}")
                nc.sync.dma_start(
                    xf,
                    xt_h[ic * P:(ic + 2) * P, c0:c0 + 512].rearrange(
                        "(c p) q -> p c q", p=P),
                )
                if own:
                    # 16-bit copy only; blocks 0-1 on ACT, 2-3 on DVE so
                    # the ACT Z0 window also fits the wv casts. The fp8
                    # hi/lo pair is derived from XT16 in the Z1 window.
                    if xb < 2:
                        nc.scalar.activation(
                            XT16[:, ic:ic + 2, c0:c0 + 512], xf,
                            mybir.ActivationFunctionType.Copy)
                    else:
                        nc.vector.tensor_copy(
                            XT16[:, ic:ic + 2, c0:c0 + 512], xf)
                else:
                    # hi fp8 on ACT, lo residual on DVE (straight from fp32)
                    nc.scalar.activation(
                        X8[:, ic:ic + 2, 0, c0:c0 + 512], xf,
                        mybir.ActivationFunctionType.Copy)
                    nc.vector.tensor_sub(
                        X8[:, ic:ic + 2, 1, c0:c0 + 512], xf,
                        X8[:, ic:ic + 2, 0, c0:c0 + 512])

        def own_x8(xb):
            # own-half fp8 hi/lo from the resident XT16 (x8 reconstructs
            # fp16(x); the extra fp16 rounding is ~0.05%, negligible).
            # Both on DVE, paced through the Z1 window where DVE is light.
            c0 = xb * 512
            nc.vector.tensor_copy(
                X8[:, :, 0, c0:c0 + 512], XT16[:, :, c0:c0 + 512])
            nc.vector.tensor_sub(
                X8[:, :, 1, c0:c0 + 512], XT16[:, :, c0:c0 + 512],
                X8[:, :, 0, c0:c0 + 512])

        ps_m = [psum_a.tile([P, JH], F32, name=f"psm{ic}_{it}",
                            tag="a") for ic in range(DC)]
        for oc in range(DC):
            wkf = wload.tile([P, 1, JH], F32, tag="wkld", bufs=2,
                             name=f"wkf{it}")
            nc.sync.dma_start(
                wkf,
                wkh_h[oc * P:(oc + 1) * P, :].rearrange(
                    "(c p) j -> p c j", p=P),
            )
            nc.vector.tensor_copy(wk_sb[:, oc:oc + 1, :], wkf)
            # each Wq o-chunk is consumed immediately — rotate a small
            # staging tile instead of keeping the full fp16 Wq in SBUF.
            # The first chunk loads in 4 column pieces so the PE's first
            # matmul issues ~2us earlier (the load+cast chain is mostly
            # fixed overhead).
            wq_rot = wqkp.tile([P, D_], FP16, tag="wqr", bufs=2,
                               name=f"wqr{it}")
            npc = 1
            for pc in range(npc):
                w0 = pc * (D_ // npc)
                w1 = (pc + 1) * (D_ // npc)
                wqf = wload.tile([P, D_ // npc], F32, tag="wld", bufs=3,
                                 name=f"wqf{it}")
                nc.sync.dma_start(wqf, wq_h[oc * P:(oc + 1) * P, w0:w1])
                nc.vector.tensor_copy(wq_rot[:, w0:w1], wqf)
                for ic in range(w0 // P, w1 // P):
                    nc.tensor.matmul(
                        ps_m[ic],
                        wq_rot[:, ic * P:(ic + 1) * P],
                        wk_sb[:, oc, :],
                        start=(oc == 0),
                        stop=(oc == DC - 1),
                    )

        # drain own M-half into static local cols 0:JH (the DRAM stores
        # are emitted later, after the x-own loads, so this 3MB of
        # M-collective traffic never delays the Z0-pacing x stream)
        for ic in range(DC):
            nc.vector.tensor_copy(m_sb[:, ic, 0:JH], ps_m[ic])

        # u = bq^T Wk from the resident wk_sb, accumulated into freshly
        # drained M banks (no pool swap: PE follows the M drains with ~no
        # idle)
        for oc in range(DC):
            for jc in range(DC // 2):
                nc.tensor.matmul(
                    ps_m[jc][:, 0:1],
                    wk_sb[:, oc, jc * P:(jc + 1) * P],
                    bqTb[:, oc:oc + 1],
                    start=(oc == 0),
                    stop=(oc == DC - 1),
                )
        u16 = up.tile([P, DC], FP16, name=f"u16{it}")
        for jc in range(DC // 2):
            nc.vector.tensor_copy(u32[:, jc:jc + 1], ps_m[jc][:, 0:1])
        nc.vector.tensor_copy(u16[:, 0:DC // 2], u32[:, 0:DC // 2])

        # x block 0 16-bit cast on the (idle) ACT engine, parallel with
        # the M drains and u copies on the DVE
        x_block(0, own=True, act_cast=True)

        wqk.close()  # wq/wk SBUF freed before the x stream peaks

        proj_out = actx.enter_context(
            tc.tile_pool(name=f"proj_out{it}", bufs=2))

        nc.gpsimd.collective_compute(
            "AllGather", mybir.AluOpType.bypass,
            replica_groups=PAIR_GROUPS,
            ins=[M_loc[:, :]], outs=[M_gath[:, :, :]],
        )

        for xb in range(1, NXB):
            x_block(xb, own=True)

        # Z^T = M^T x_own^T (unscaled: Z ~ N(0,1), ideal e4m3 range; the
        # 1/32 score scale folds into the exp). Drain: ACT z16 =
        # ps + u[jc] -> fp16 staging; ACT z8 = fp8(z16); DVE zlo = z16-z8.
        # Own j-half first (never waits on the M gather).
        z16p = actx.enter_context(tc.tile_pool(name=f"z16p{it}", bufs=6))

        def z_half(jh, post_qb=None):
            for qb in range(NXB):
                q0 = qb * 512
                for jc in range(jh * DC // 2, (jh + 1) * DC // 2):
                    ps = psum_a.tile([P, 512], F32, name=f"psz{it}", tag="a")
                    for ic in range(DC):
                        nc.tensor.matmul(
                            ps,
                            m_sb[:, ic, jc * P:(jc + 1) * P],
                            XT16[:, ic, q0:q0 + 512],
                            start=(ic == 0),
                            stop=(ic == DC - 1),
                        )
                    z16 = z16p.tile([P, 512], FP16, name=f"z16_{it}",
                                    tag="z16", bufs=6)
                    nc.scalar.activation(
                        z16, ps, mybir.ActivationFunctionType.Identity,
                        bias=u32[:, jc:jc + 1],
                    )
                    nc.scalar.activation(
                        Z8[:, jc, 1, q0:q0 + 512], z16,
                        mybir.ActivationFunctionType.Copy)
                    nc.vector.tensor_sub(
                        Z8[:, jc, 0, q0:q0 + 512], z16,
                        Z8[:, jc, 1, q0:q0 + 512])
                if post_qb is not None:
                    post_qb(qb)

        # Z0 with the Wv loads + fp16 casts paced two chunks per qb (the
        # ACT casts land in-queue between Z drains, all data-ready by then)
        wv_sb = mtp.tile([P, DC, D_], FP16, name=f"wv_sb{it}")

        def wv_pair(qb):
            for ic in (2 * qb, 2 * qb + 1):
                wf = wload.tile([P, 2, JH], F32, tag="wld", bufs=3,
                                name=f"wvf{it}")
                nc.sync.dma_start(
                    wf, wv_h[ic * P:(ic + 1) * P, :].rearrange(
                        "p (c j) -> p c j", j=JH))
                nc.scalar.activation(
                    wv_sb[:, ic, :].rearrange("p (c j) -> p c j", j=JH), wf,
                    mybir.ActivationFunctionType.Copy)

        z_half(0, post_qb=wv_pair)

        # M payload stores (SP ring, queued behind the x-own stream)
        for ic in range(DC):
            nc.sync.dma_start(M_loc[:, ic * JH:(ic + 1) * JH],
                              m_sb[:, ic, 0:JH])
        nc.sync.dma_start(M_loc[:, MW:MW + 4], u16[:, 0:DC // 2])

        # partner M-half -> local cols JH:2JH (the local j-permutation is
        # [own, partner] on every core, mirrored in the host inputs). The
        # partner's payload i-chunks are in ITS local order (halves swapped
        # vs ours), so payload chunks [4:8] are our chunks 0:4 and vice
        # versa. Read back as two 1MB transfers on the SP ring.
        mg = M_gath[bass.ds(partner, 1), :, :][0]
        nc.sync.dma_start(
            m_sb[:, 0:DC // 2, JH:D_],
            mg[:, DC // 2 * JH:DC * JH].rearrange("p (c j) -> p c j", j=JH),
        )
        nc.sync.dma_start(
            m_sb[:, DC // 2:DC, JH:D_],
            mg[:, 0:DC // 2 * JH].rearrange("p (c j) -> p c j", j=JH),
        )
        ug = up.tile([P, DC // 2], FP16, name=f"ug{it}")
        nc.sync.dma_start(ug, mg[:, MW:MW + 4])
        nc.vector.tensor_copy(u32[:, DC // 2:DC], ug)


        # V projection -> v8/vlo fp8 pair -> DRAM -> pair AllGather, with
        # the partner-half x fp8 casts interleaved (one block per 4 kt) so
        # neither stream head-of-line blocks the other. The V collective
        # chain (stores, gather, phase-B readback) hides behind scores.
        if True:
            for kt in range(KCL):
                v8t = proj_out.tile([P, D_], F8, tag="v8", bufs=2,
                                    name=f"v8t{it}")
                vlot = proj_out.tile([P, D_], F8, tag="vlo", bufs=2,
                                     name=f"vlot{it}")
                for dh in range(D_ // 512):
                    ps = psum_a.tile([P, 512], F32, name=f"psv{it}",
                                     tag="a")
                    for ic in range(DC):
                        nc.tensor.matmul(
                            ps,
                            XT16[:, ic, kt * P:(kt + 1) * P],
                            wv_sb[:, ic, dh * 512:(dh + 1) * 512],
                            start=(ic == 0),
                            stop=(ic == DC - 1),
                        )
                    # hi/lo fp8 pair straight from PSUM (exact fp32 V,
                    # no fp16 staging; bv moves to the phase-B drain)
                    d0 = dh * 512
                    nc.scalar.activation(
                        v8t[:, d0:d0 + 512], ps,
                        mybir.ActivationFunctionType.Copy)
                    nc.vector.tensor_sub(vlot[:, d0:d0 + 512], ps,
                                         v8t[:, d0:d0 + 512])
                nc.sync.dma_start(V_loc[:, kt, 1, :], v8t)
                nc.sync.dma_start(V_loc[:, kt, 0, :], vlot)
                if kt % 4 == 3:
                    x_block(NXB + kt // 4, own=False)
        nc.gpsimd.collective_compute(
            "AllGather", mybir.AluOpType.bypass,
            replica_groups=PAIR_GROUPS,
            ins=[V_loc[:, :, :, :]], outs=[V_gath[:, :, :, :, :]],
        )

        # Z1 after V so the V collective chain hides behind Z1 + scores.
        # The own-half fp8 casts are front-loaded into the first two qb
        # slots so the X8 tail never delays the first scores matmul.
        def z1_hook(qb):
            if qb == 0:
                own_x8(0)
                own_x8(1)
            elif qb == 1:
                own_x8(2)
                own_x8(3)

        z_half(1, post_qb=z1_hook)

        # ---- phase B
        actx.close()
        # V2: hi/lo pairs with ones-columns at dv=1024..1032 in the v8
        # half (vlo half zeroed): the A.V matmul produces the softmax
        # denominator sum(a8)+sum(alo) for free in its third chunk. Key
        # order [own, partner] matches X8. The own half is read straight
        # from local DRAM, never waiting on the collective.
        assert D_ == 1024
        vres = itctx.enter_context(tc.tile_pool(name=f"vres{it}", bufs=1))
        # bv broadcast, applied at the output drain (attention is linear
        # in V and bv is key-constant: out = att(V) + bv exactly)
        bvb = vres.tile([P, D_], F32, name=f"bvb{it}")
        nc.gpsimd.dma_start(bvb, bv_h[None, :].to_broadcast([P, D_]))
        V_sb = vres.tile([P, KC, 2, DV], F8, name=f"V_sb{it}")
        nc.vector.memset(V_sb[:, :, 0, D_:DV], 0.0)
        nc.vector.memset(V_sb[:, :, 1, D_:DV], 1.0)
        nc.sync.dma_start(V_sb[:, 0:KCL, :, :D_], V_loc[:, :, :, :])
        nc.sync.dma_start(
            V_sb[:, KCL:2 * KCL, :, :D_],
            V_gath[bass.ds(partner, 1), :, :, :, :][0],
        )
        alpha = itctx.enter_context(tc.tile_pool(name=f"alpha{it}", bufs=1))
        a16p = itctx.enter_context(tc.tile_pool(name=f"a16p{it}", bufs=3))
        outp = itctx.enter_context(tc.tile_pool(name=f"outp{it}", bufs=2))
        recipp = itctx.enter_context(tc.tile_pool(name=f"recipp{it}", bufs=4))
        psum_s = itctx.enter_context(
            tc.tile_pool(name=f"psum_s{it}", bufs=2, space="PSUM"))
        psum_av = itctx.enter_context(
            tc.tile_pool(name=f"psum_av{it}", bufs=6, space="PSUM"))

        for blk in range(NBLK):
            q0 = blk * qblk
            # A2 pair dim: 0 = a8, 1 = alo (lhsT (a8,alo) x rhs (vlo,v8)
            # = a8.vlo + alo.v8 in one DR instruction)
            A2 = alpha.tile([P, KC, 2, qblk], F8, name=f"A2_{it}")
            for kc in range(KC):
                ps = psum_s.tile([P, qblk], F32, name=f"ps_s{it}")
                k0 = kc * P
                for icp in range(DC // 2):
                    ic = icp * 2
                    nc.tensor.matmul(
                        ps,
                        X8[:, ic:ic + 2, 0, k0:k0 + P],
                        Z8[:, ic:ic + 2, 1, q0:q0 + qblk],
                        start=(ic == 0), stop=False,
                        perf_mode=mybir.MatmulPerfMode.DoubleRow,
                    )
                for ic in range(DC):
                    nc.tensor.matmul(
                        ps,
                        X8[:, ic, :, k0:k0 + P],
                        Z8[:, ic, :, q0:q0 + qblk],
                        start=False, stop=(ic == DC - 1),
                        perf_mode=mybir.MatmulPerfMode.DoubleRow,
                    )
                a16 = a16p.tile([P, qblk], FP16, name=f"a16_{it}",
                                tag="a16", bufs=3)
                nc.scalar.activation(
                    a16, ps, mybir.ActivationFunctionType.Exp,
                    bias=ebias_t, scale=scale,
                )
                nc.vector.tensor_copy(A2[:, kc, 0, :], a16)
                nc.vector.tensor_sub(A2[:, kc, 1, :], a16, A2[:, kc, 0, :])
            # qi-outer: each query-tile's drain overlaps the next tile's
            # matmuls, shrinking the end-of-block (and end-of-kernel) tail
            for qt_l in range(QT_PER_BLK):
                last = (blk == NBLK - 1 and qt_l == QT_PER_BLK - 1)
                avs = [
                    psum_av.tile([P, CH], F32, name=f"av{i}_{it}", tag="av")
                    for i in range(3)
                ]
                qs = qt_l * P
                rc = recipp.tile([P, 1], F32, name=f"rc{it}")
                out_t = outp.tile([P, D_], F32, name=f"out_t{it}")
                row0 = (blk * QT_PER_BLK + qt_l) * P

                def av_matmuls(chs, c0=0, c1=CH):
                    first = True
                    for kcp in range(KC // 2):
                        kc = kcp * 2
                        for ch in chs:
                            nc.tensor.matmul(
                                avs[ch][:, c0:c1],
                                A2[:, kc:kc + 2, 0, qs:qs + P],
                                V_sb[:, kc:kc + 2, 1,
                                     ch * CH + c0:ch * CH + c1],
                                start=first, stop=False,
                                perf_mode=mybir.MatmulPerfMode.DoubleRow,
                            )
                        first = False
                    for kc in range(KC):
                        lastk = (kc == KC - 1)
                        for ch in chs:
                            nc.tensor.matmul(
                                avs[ch][:, c0:c1],
                                A2[:, kc, :, qs:qs + P],
                                V_sb[:, kc, :, ch * CH + c0:ch * CH + c1],
                                start=False, stop=lastk,
                                perf_mode=mybir.MatmulPerfMode.DoubleRow,
                            )

                def drain(ch):
                    # out = av*rc + bv in one DVE op
                    w = CH if ch < 2 else D_ - 2 * CH
                    c0 = ch * CH
                    nc.vector.affine_then_add(
                        out_t[:, c0:c0 + w], avs[ch][:, :w],
                        bvb[:, c0:c0 + w], rc, 0.0)

                if last:
                    # denominator chunk first: its reciprocal + each chunk's
                    # drain + store hide behind the next chunk's matmuls,
                    # shrinking the end-of-kernel tail; the final chunk runs
                    # as two half-width psum regions so even its first
                    # half's drain + store hides behind the second half
                    av_matmuls([2])
                    nc.vector.reciprocal(
                        rc, avs[2][:, D_ - 2 * CH:D_ - 2 * CH + 1])
                    drain(2)
                    nc.sync.dma_start(
                        out_h[row0:row0 + P, 2 * CH:D_],
                        out_t[:, 2 * CH:D_])
                    av_matmuls([0])
                    drain(0)
                    nc.sync.dma_start(
                        out_h[row0:row0 + P, 0:CH], out_t[:, 0:CH])
                    av_matmuls([1])
                    drain(1)
                    nc.sync.dma_start(
                        out_h[row0:row0 + P, CH:2 * CH], out_t[:, CH:2 * CH])
                else:
                    av_matmuls([0, 1, 2])
                    # denominator = column 1024 = chunk 2, local col 336
                    nc.vector.reciprocal(
                        rc, avs[2][:, D_ - 2 * CH:D_ - 2 * CH + 1])
                    for ch in range(3):
                        drain(ch)
                    nc.sync.dma_start(out_h[row0:row0 + P, :], out_t)


_CACHED_NC = None


def make_in_maps(x, Wq, bq, Wk, bk, Wv, bv, sq=None):
    sq = SQ if sq is None else sq
    x = np.asarray(x, dtype=np.float32)
    Wq = np.asarray(Wq, np.float32)
    Wk = np.asarray(Wk, np.float32)
    WvT = np.asarray(Wv, np.float32).T
    in_maps = []
    for c in range(N_CORES):
        b, h = divmod(c, 2)
        # local contraction-dim order = [own j-half, partner j-half]
        if h == 0:
            perm = np.arange(D)
        else:
            perm = np.r_[np.arange(D // 2, D), np.arange(0, D // 2)]
        # full xT: own query-half keys first, then the partner half, both
        # row-permuted into this core's local contraction order
        xb = np.concatenate(
            [x[b][h * sq:(h + 1) * sq], x[b][(1 - h) * sq:(2 - h) * sq]],
            axis=0)
        in_maps.append({
            "xT": np.ascontiguousarray(xb.T[perm]),
            "Wq": np.ascontiguousarray(Wq[:, perm]),
            "WkH": np.ascontiguousarray(Wk[:, h * (D // 2):(h + 1) * (D // 2)]),
            "WvT": np.ascontiguousarray(WvT[perm]),
            "bqT": np.ascontiguousarray(
                np.asarray(bq, np.float32).reshape(D // P, P).T),
            "bv": np.asarray(bv, np.float32),
        })
    return in_maps


def gather_out(results):
    out = np.empty((B, S_FULL, D), np.float32)
    for c in range(N_CORES):
        b, h = divmod(c, 2)
        out[b, h * SQ:(h + 1) * SQ, :] = results[c]["out"]
    return out


def kernel(x, Wq, bq, Wk, bk, Wv, bv):
    from concourse.bass_utils import run_bass_kernel_spmd

    global _CACHED_NC
    if _CACHED_NC is None:
        _CACHED_NC = build_module_cc(S_FULL, SQ, D)
    nc = _CACHED_NC

    in_maps = make_in_maps(x, Wq, bq, Wk, bk, Wv, bv)
    # the device pool occasionally wedges transiently on cold runs
    # (NRT_EXEC_UNIT_UNRECOVERABLE, clears on retry) — retry before failing,
    # resetting the jax/PJRT backend so the retry gets a fresh client
    last_exc = None
    for attempt in range(3):
        try:
            res = run_bass_kernel_spmd(nc, in_maps, list(range(N_CORES)))
            return gather_out(res.results)
        except Exception as e:  # noqa: BLE001 - deliberate broad retry
            last_exc = e
            try:
                import time
                import jax
                time.sleep(2)
                jax.clear_caches()
                jax.extend.backend.clear_backends()
            except Exception:
                pass
    raise last_exc


# revision 68
# speedup vs baseline: 1.0013x; 1.0013x over previous
"""Trainium2 Bass kernel for nn_AttentionLayer (B=4, S=4096, D=1024, fp32).

Sharding: 8 cores = 4 batches x 2 query-halves. Each core receives the
TRANSPOSED x rows of its own query half plus Wq (i-columns permuted), its
own j-half of Wk, Wv^T (i-rows permuted), bq and bv — all host-side layout
marshaling only (transpose / slice / permute / concatenate; values and
dtypes unchanged). Each core receives the FULL 4096-key xT (own query-half
keys first, then the partner half, in its own local order), so no x
collective exists; core pairs (same batch) exchange only M-halves and V
halves with local-output AllGathers. Each core computes single-head
attention for its query half and writes a [2048, 1024] fp32 slice; the
host gathers slices into [4, 4096, 1024].

Key algebraic restructure vs the direct formulation: softmax is invariant
to per-query score shifts, so with M := Wq^T Wk and u := bq^T Wk,

    S ~ Z x_all^T  (mod per-query shifts),   Z := x_own M + 1 (x) u

reproduces softmax(QK^T) EXACTLY (the bk and bq.bk cross terms are
per-query constants and cancel — bk is never needed on device). This
eliminates the K projection.

Precision/speed structure (v2): the two dominant matmuls (scores and
alpha.V — 78% of PE cycles in the bf16 formulation) run as fp8e4
DoubleRow matmuls (2 contraction chunks per instruction, 0.5 cycles/row:
4x the bf16 rate). Single-fp8 operands would blow the 2e-2 error budget
(measured ~2-5e-2 via numpy emulation), so every fp8 operand is a
compensated hi+lo pair (lo = fp8(x - fp8(x)), effective precision ~bf16)
and each matmul runs 3 of the 4 hi/lo cross terms (lo*lo dropped):

    scores: x8.z8 (+) x8.zlo (+) xlo.z8     [12 DR instrs / 512-q block]
    A.V:    a8.v8 (+) a8.vlo (+) alo.v8     [48 DR instrs / q-tile.chunk]

at 0.75x the bf16 cycle cost per matmul. The hi/lo pairs are packed on a
middle tile dim so one DoubleRow instruction computes both cross terms:
X8[:,ic,{x8,xlo},:] x Z8[:,ic,{zlo,z8},:] = x8.zlo + xlo.z8. All other
matmuls (M, Z and V projections) stay 16-bit, upgraded bf16 -> fp16
(same PE rate, 8x less rounding). The V hi+lo fp8 pair is the collective
payload (same bytes as one bf16 V). Z is unscaled (std ~1, ideal for
e4m3); the 1/32 score scale and a -1.5 shift (to keep exp under e4m3
max) fold into the ACT exp. alpha's ones-column denominator rides in the
v8 half (vlo ones-cols are 0), so den = sum(a8)+sum(alo) matches the
numerator's effective alpha exactly. Numpy emulation of this exact
rounding chain: rel err 1.7e-3 vs the 2e-2 gate (bf16 baseline: 6.4e-3);
measured on the hardware path: 2.16e-3.

Per-core PE work: M 33k + Z 131k + V 131k + scores 393k + AV 396k
~= 1086k cycles (bf16 baseline: 1348k). TimelineSim (the perf metric,
with collectives replaced by equivalent-volume local DMAs as in
test.py): 489494 ns vs the 594897 ns bf16 baseline. Phase B (scores+AV,
~331us) runs at ~100% PE occupancy; phase A (~154us) is bounded by the
26MB input-load stream on the single shared DMA device plus the
fp8-pair cast work on ACT/DVE, orchestrated via FIFO-emission-order
scheduling (see below).

Per-core program (SPMD, identical on all cores). DMA ring assignment:
sync(SP) = input loads + V stores + collective readbacks + out stores,
scalar(ACT) = M stores, gpsimd(Pool) = collective triggers. Engine
split for the drain/cast work: ACT = z16/z8/x8/v8 casts + exp + AV
drains; DVE = weight/x16 casts, lo-residual subs, v16 bias add, a8/alo.
  phase A: u = bq^T WkH chases the Wk DMAs; M-half = Wq^T WkH chases the
           Wq DMAs in 8 PSUM banks; M+u pair-AllGather (fp16 payload);
           x loads -> XT16 (own half) + X8 hi/lo (all keys); Z^T = M^T
           x_own^T -> Z8 hi/lo, SBUF-resident; V = x_own Wv^T + bv ->
           v8/vlo -> DRAM -> pair AllGather.
  phase B: V2 resident in SBUF (vlo,v8 pairs + ones-cols). Per 512-query
           block: 12 DR matmuls per key chunk -> exp on ACT -> a8/alo;
           out = (A.V)/den accumulated over 16 key-chunk-pairs x 3
           output chunks of 344; qi-outer AV ordering overlaps each
           query-tile's drain with the next tile's matmuls; the final
           tile runs denominator-chunk-first with split stores.
"""

import math
from contextlib import ExitStack

import numpy as np

import concourse.bass as bass
import concourse.tile as tile
from concourse import bacc, mybir

F32 = mybir.dt.float32
FP16 = mybir.dt.float16
F8 = mybir.dt.float8e4
P = 128

# Full-problem constants (hardcoded; harness provides matching inputs).
B, S_FULL, D = 4, 4096, 1024
N_CORES = 8
SQ = S_FULL // 2  # query rows per core

PAIR_GROUPS = [[0, 1], [2, 3], [4, 5], [6, 7]]

EBIAS = -1.5  # folded into exp; cancels in softmax, keeps a8 < e4m3 max


def build_module_cc(S, SQ_, D_, qblk=512, niter=1):
    """Build the per-core Bass program. S = key rows, SQ_ = query rows."""
    assert S == 2 * SQ_
    nc = bacc.Bacc(None, num_devices=N_CORES)
    DC = D_ // P          # 128-chunks of the model dim (8)

    xt_h = nc.dram_tensor("xT", [D_, S], F32, kind="ExternalInput")
    wq_h = nc.dram_tensor("Wq", [D_, D_], F32, kind="ExternalInput")
    wkh_h = nc.dram_tensor("WkH", [D_, D_ // 2], F32, kind="ExternalInput")
    wv_h = nc.dram_tensor("WvT", [D_, D_], F32, kind="ExternalInput")
    bqt_h = nc.dram_tensor("bqT", [P, D_ // P], F32, kind="ExternalInput")
    bv_h = nc.dram_tensor("bv", [D_], F32, kind="ExternalInput")
    out_h = nc.dram_tensor("out", [SQ_, D_], F32, kind="ExternalOutput")

    with tile.TileContext(nc) as tc, ExitStack() as ctx:
        consts = ctx.enter_context(tc.tile_pool(name="consts", bufs=1))
        dram = ctx.enter_context(tc.tile_pool(name="dram", bufs=1, space="DRAM"))

        # bq arrives host-striped as [P, DC] (element (p,c) = bq[c*128+p]);
        # loaded on the Pool ring so the SP load stream starts at t=0
        bqT = consts.tile([P, DC], F32)
        nc.gpsimd.dma_start(bqT, bqt_h[:, :])
        bqTb = consts.tile([P, DC], FP16)
        nc.vector.tensor_copy(bqTb, bqT)
        ebias_t = consts.tile([P, 1], F32)
        nc.vector.memset(ebias_t, EBIAS)
        # warm the ACT function tables (Exp for phase B, Identity for the
        # Z drains) at t~0 while ACT is idle, so no ACT_TABLE_LOAD lands
        # on the critical path later; the scrap result is never read
        warm = consts.tile([P, 1], F32)
        nc.scalar.activation(warm, ebias_t,
                             mybir.ActivationFunctionType.Exp)
        nc.scalar.activation(warm, ebias_t,
                             mybir.ActivationFunctionType.Identity,
                             bias=ebias_t)
        pid = nc.partition_id()

        for it in range(niter):
            _emit_iteration(
                nc, tc, dram, it, S, SQ_, D_, qblk,
                xt_h, wq_h, wkh_h, wv_h, bv_h, out_h,
                bqTb, ebias_t, pid,
            )

    nc.finalize()
    return nc


def _emit_iteration(nc, tc, dram, it, S, SQ_, D_, qblk,
                    xt_h, wq_h, wkh_h, wv_h, bv_h, out_h,
                    bqTb, ebias_t, pid):
    DC = D_ // P
    JH = D_ // 2          # j-half width (512)
    KC = S // P           # gathered key chunks (32)
    KCL = SQ_ // P        # local key chunks (16)
    NBLK = SQ_ // qblk    # query blocks (4)
    QT_PER_BLK = qblk // P
    scale = 1.0 / math.sqrt(D_)
    MW = DC * JH          # M-half payload cols (4096); + 4 for u
    DV = D_ + 8           # V cols + ones-column pad (1032 = 3*344)
    CH = 344

    with ExitStack() as itctx:
        x8p = itctx.enter_context(tc.tile_pool(name=f"x8p{it}", bufs=1))
        z8p = itctx.enter_context(tc.tile_pool(name=f"z8p{it}", bufs=1))
        up = itctx.enter_context(tc.tile_pool(name=f"up{it}", bufs=1))

        actx = ExitStack()
        ktp = actx.enter_context(tc.tile_pool(name=f"ktp{it}", bufs=1))
        mtp = actx.enter_context(tc.tile_pool(name=f"mtp{it}", bufs=1))
        wload = actx.enter_context(tc.tile_pool(name=f"wload{it}", bufs=3))
        xload = actx.enter_context(tc.tile_pool(name=f"xload{it}", bufs=3))

        consts_a = actx.enter_context(
            tc.tile_pool(name=f"consts_a{it}", bufs=1))
        # bv broadcast to all partitions, loaded at t=0 on the Pool ring
        # while the DMA device is otherwise idle
        bvb = consts_a.tile([P, D_], F32, name=f"bvb{it}")
        nc.gpsimd.dma_start(bvb, bv_h[None, :].to_broadcast([P, D_]))

        M_loc = dram.tile([P, MW + 4], FP16, name=f"M_loc{it}", tag=f"ML{it}")
        M_gath = dram.tile([2, P, MW + 4], FP16, name=f"M_gath{it}",
                           tag=f"MG{it}")
        V_loc = dram.tile([P, KCL, 2, D_], F8, name=f"V_loc{it}",
                          tag=f"VL{it}")
        V_gath = dram.tile([2, P, KCL, 2, D_], F8, name=f"V_gath{it}",
                           tag=f"VG{it}")

        # XT16: own-half keys only (Z and V projections); 16-bit.
        XT16 = ktp.tile([P, DC, SQ_], FP16, name=f"XT16{it}")
        # X8: hi/lo fp8 pair for ALL 4096 keys (scores lhsT). Pair dim:
        # index 0 = x8 = fp8(x), 1 = xlo = fp8(x - x8).
        X8 = x8p.tile([P, DC, 2, S], F8, name=f"X8{it}")
        # Z8: hi/lo fp8 pair, SBUF-resident. Pair dim: 0 = zlo, 1 = z8
        # (so X8[:,ic,:,:] x Z8[:,ic,:,:] = x8.zlo + xlo.z8 in one DR).
        Z8 = z8p.tile([P, DC, 2, SQ_], F8, name=f"Z8{it}")

        wqk = ExitStack()
        wqkp = wqk.enter_context(tc.tile_pool(name=f"wqkp{it}", bufs=1))
        wk_sb = wqkp.tile([P, DC, JH], FP16, name=f"wk_sb{it}")
        m_sb = mtp.tile([P, DC, D_], FP16, name=f"m_sb{it}")
        u32 = up.tile([P, DC], F32, name=f"u32{it}")

        # ---- phase A
        partner = (pid + 1) % 2

        # One shared phase-A PSUM pool: a single [P, 512] tag ring over
        # all 8 banks serves M, u, Z0, V and Z1 with no pool-swap stalls.
        psum_a = actx.enter_context(
            tc.tile_pool(name=f"psum_a{it}", bufs=8, space="PSUM"))

        # M-half = Wq^T Wk[:, own-j-half] chases interleaved Wk/Wq chunk
        # loads on the ACT DMA ring (each o-chunk of M needs only that
        # o-chunk of Wk and Wq, so the PE starts ~3us in); u = bq^T Wk
        # runs after M from the resident wk_sb into drained M banks.
        NXB = SQ_ // 512

        def x_block(xb, own, act_cast=False):
            c0 = xb * 512
            for icp in range(DC // 2):
                ic = icp * 2
                xf = xload.tile([P, 2, 512], F32, tag="ld", bufs=3,
                                name=f"xf{it}")
                nc.sync.dma_start(
                    xf,
                    xt_h[ic * P:(ic + 2) * P, c0:c0 + 512].rearrange(
                        "(c p) q -> p c q", p=P),
                )
                if own:
                    # 16-bit copy only; blocks 0-1 on ACT, 2-3 on DVE so
                    # the ACT Z0 window also fits the wv casts. The fp8
                    # hi/lo pair is derived from XT16 in the Z1 window.
                    if xb < 2:
                        nc.scalar.activation(
                            XT16[:, ic:ic + 2, c0:c0 + 512], xf,
                            mybir.ActivationFunctionType.Copy)
                    else:
                        nc.vector.tensor_copy(
                            XT16[:, ic:ic + 2, c0:c0 + 512], xf)
                else:
                    # hi fp8 on ACT, lo residual on DVE (straight from fp32)
                    nc.scalar.activation(
                        X8[:, ic:ic + 2, 0, c0:c0 + 512], xf,
                        mybir.ActivationFunctionType.Copy)
                    nc.vector.tensor_sub(
                        X8[:, ic:ic + 2, 1, c0:c0 + 512], xf,
                        X8[:, ic:ic + 2, 0, c0:c0 + 512])

        def own_x8(xb):
            # own-half fp8 hi/lo from the resident XT16 (x8 reconstructs
            # fp16(x); the extra fp16 rounding is ~0.05%, negligible).
            # Both on DVE, paced through the Z1 window where DVE is light.
            c0 = xb * 512
            nc.vector.tensor_copy(
                X8[:, :, 0, c0:c0 + 512], XT16[:, :, c0:c0 + 512])
            nc.vector.tensor_sub(
                X8[:, :, 1, c0:c0 + 512], XT16[:, :, c0:c0 + 512],
                X8[:, :, 0, c0:c0 + 512])

        ps_m = [psum_a.tile([P, JH], F32, name=f"psm{ic}_{it}",
                            tag="a") for ic in range(DC)]
        for oc in range(DC):
            wkf = wload.tile([P, 1, JH], F32, tag="wkld", bufs=2,
                             name=f"wkf{it}")
            nc.sync.dma_start(
                wkf,
                wkh_h[oc * P:(oc + 1) * P, :].rearrange(
                    "(c p) j -> p c j", p=P),
            )
            nc.vector.tensor_copy(wk_sb[:, oc:oc + 1, :], wkf)
            # each Wq o-chunk is consumed immediately — rotate a small
            # staging tile instead of keeping the full fp16 Wq in SBUF.
            # The first chunk loads in 4 column pieces so the PE's first
            # matmul issues ~2us earlier (the load+cast chain is mostly
            # fixed overhead).
            wq_rot = wqkp.tile([P, D_], FP16, tag="wqr", bufs=2,
                               name=f"wqr{it}")
            npc = 1
            for pc in range(npc):
                w0 = pc * (D_ // npc)
                w1 = (pc + 1) * (D_ // npc)
                wqf = wload.tile([P, D_ // npc], F32, tag="wld", bufs=3,
                                 name=f"wqf{it}")
                nc.sync.dma_start(wqf, wq_h[oc * P:(oc + 1) * P, w0:w1])
                nc.vector.tensor_copy(wq_rot[:, w0:w1], wqf)
                for ic in range(w0 // P, w1 // P):
                    nc.tensor.matmul(
                        ps_m[ic],
                        wq_rot[:, ic * P:(ic + 1) * P],
                        wk_sb[:, oc, :],
                        start=(oc == 0),
                        stop=(oc == DC - 1),
                    )

        # drain own M-half into static local cols 0:JH (the DRAM stores
        # are emitted later, after the x-own loads, so this 3MB of
        # M-collective traffic never delays the Z0-pacing x stream)
        for ic in range(DC):
            nc.vector.tensor_copy(m_sb[:, ic, 0:JH], ps_m[ic])

        # u = bq^T Wk from the resident wk_sb, accumulated into freshly
        # drained M banks (no pool swap: PE follows the M drains with ~no
        # idle)
        for oc in range(DC):
            for jc in range(DC // 2):
                nc.tensor.matmul(
                    ps_m[jc // 2][:, jc % 2:jc % 2 + 1],
                    wk_sb[:, oc, jc * P:(jc + 1) * P],
                    bqTb[:, oc:oc + 1],
                    start=(oc == 0),
                    stop=(oc == DC - 1),
                )
        u16 = up.tile([P, DC], FP16, name=f"u16{it}")
        for jc in range(DC // 2):
            nc.vector.tensor_copy(u32[:, jc:jc + 1],
                                  ps_m[jc // 2][:, jc % 2:jc % 2 + 1])
        nc.vector.tensor_copy(u16[:, 0:DC // 2], u32[:, 0:DC // 2])

        # x block 0 16-bit cast on the (idle) ACT engine, parallel with
        # the M drains and u copies on the DVE
        x_block(0, own=True, act_cast=True)

        wqk.close()  # wq/wk SBUF freed before the x stream peaks

        proj_out = actx.enter_context(
            tc.tile_pool(name=f"proj_out{it}", bufs=2))

        nc.gpsimd.collective_compute(
            "AllGather", mybir.AluOpType.bypass,
            replica_groups=PAIR_GROUPS,
            ins=[M_loc[:, :]], outs=[M_gath[:, :, :]],
        )

        for xb in range(1, NXB):
            x_block(xb, own=True)

        # Z^T = M^T x_own^T (unscaled: Z ~ N(0,1), ideal e4m3 range; the
        # 1/32 score scale folds into the exp). Drain: ACT z16 =
        # ps + u[jc] -> fp16 staging; ACT z8 = fp8(z16); DVE zlo = z16-z8.
        # Own j-half first (never waits on the M gather).
        z16p = actx.enter_context(tc.tile_pool(name=f"z16p{it}", bufs=6))

        def z_half(jh, post_qb=None):
            for qb in range(NXB):
                q0 = qb * 512
                for jc in range(jh * DC // 2, (jh + 1) * DC // 2):
                    ps = psum_a.tile([P, 512], F32, name=f"psz{it}", tag="a")
                    for ic in range(DC):
                        nc.tensor.matmul(
                            ps,
                            m_sb[:, ic, jc * P:(jc + 1) * P],
                            XT16[:, ic, q0:q0 + 512],
                            start=(ic == 0),
                            stop=(ic == DC - 1),
                        )
                    z16 = z16p.tile([P, 512], FP16, name=f"z16_{it}",
                                    tag="z16", bufs=6)
                    nc.scalar.activation(
                        z16, ps, mybir.ActivationFunctionType.Identity,
                        bias=u32[:, jc:jc + 1],
                    )
                    nc.scalar.activation(
                        Z8[:, jc, 1, q0:q0 + 512], z16,
                        mybir.ActivationFunctionType.Copy)
                    nc.vector.tensor_sub(
                        Z8[:, jc, 0, q0:q0 + 512], z16,
                        Z8[:, jc, 1, q0:q0 + 512])
                if post_qb is not None:
                    post_qb(qb)

        # Z0 with the Wv loads + fp16 casts paced two chunks per qb (the
        # ACT casts land in-queue between Z drains, all data-ready by then)
        wv_sb = mtp.tile([P, DC, D_], FP16, name=f"wv_sb{it}")

        def wv_pair(qb):
            for ic in (2 * qb, 2 * qb + 1):
                wf = wload.tile([P, 2, JH], F32, tag="wld", bufs=3,
                                name=f"wvf{it}")
                nc.sync.dma_start(
                    wf, wv_h[ic * P:(ic + 1) * P, :].rearrange(
                        "p (c j) -> p c j", j=JH))
                nc.scalar.activation(
                    wv_sb[:, ic, :].rearrange("p (c j) -> p c j", j=JH), wf,
                    mybir.ActivationFunctionType.Copy)

        z_half(0, post_qb=wv_pair)

        # M payload stores (SP ring, queued behind the x-own stream)
        for ic in range(DC):
            nc.sync.dma_start(M_loc[:, ic * JH:(ic + 1) * JH],
                              m_sb[:, ic, 0:JH])
        nc.sync.dma_start(M_loc[:, MW:MW + 4], u16[:, 0:DC // 2])

        # partner M-half -> local cols JH:2JH (the local j-permutation is
        # [own, partner] on every core, mirrored in the host inputs). The
        # partner's payload i-chunks are in ITS local order (halves swapped
        # vs ours), so payload chunks [4:8] are our chunks 0:4 and vice
        # versa. Read back as two 1MB transfers on the SP ring.
        mg = M_gath[bass.ds(partner, 1), :, :][0]
        nc.sync.dma_start(
            m_sb[:, 0:DC // 2, JH:D_],
            mg[:, DC // 2 * JH:DC * JH].rearrange("p (c j) -> p c j", j=JH),
        )
        nc.sync.dma_start(
            m_sb[:, DC // 2:DC, JH:D_],
            mg[:, 0:DC // 2 * JH].rearrange("p (c j) -> p c j", j=JH),
        )
        ug = up.tile([P, DC // 2], FP16, name=f"ug{it}")
        nc.sync.dma_start(ug, mg[:, MW:MW + 4])
        nc.vector.tensor_copy(u32[:, DC // 2:DC], ug)


        # V projection -> v8/vlo fp8 pair -> DRAM -> pair AllGather, with
        # the partner-half x fp8 casts interleaved (one block per 4 kt) so
        # neither stream head-of-line blocks the other. The V collective
        # chain (stores, gather, phase-B readback) hides behind scores.
        if True:
            for kt in range(KCL):
                v8t = proj_out.tile([P, D_], F8, tag="v8", bufs=2,
                                    name=f"v8t{it}")
                vlot = proj_out.tile([P, D_], F8, tag="vlo", bufs=2,
                                     name=f"vlot{it}")
                for dh in range(D_ // 512):
                    ps = psum_a.tile([P, 512], F32, name=f"psv{it}",
                                     tag="a")
                    for ic in range(DC):
                        nc.tensor.matmul(
                            ps,
                            XT16[:, ic, kt * P:(kt + 1) * P],
                            wv_sb[:, ic, dh * 512:(dh + 1) * 512],
                            start=(ic == 0),
                            stop=(ic == DC - 1),
                        )
                    # hi/lo fp8 pair straight from PSUM (exact fp32 V,
                    # no fp16 staging; bv moves to the phase-B drain)
                    d0 = dh * 512
                    nc.scalar.activation(
                        v8t[:, d0:d0 + 512], ps,
                        mybir.ActivationFunctionType.Copy)
                    nc.vector.tensor_sub(vlot[:, d0:d0 + 512], ps,
                                         v8t[:, d0:d0 + 512])
                nc.sync.dma_start(V_loc[:, kt, 1, :], v8t)
                nc.sync.dma_start(V_loc[:, kt, 0, :], vlot)
                if kt % 4 == 3:
                    x_block(NXB + kt // 4, own=False)
        nc.gpsimd.collective_compute(
            "AllGather", mybir.AluOpType.bypass,
            replica_groups=PAIR_GROUPS,
            ins=[V_loc[:, :, :, :]], outs=[V_gath[:, :, :, :, :]],
        )

        # Z1 after V so the V collective chain hides behind Z1 + scores.
        # The own-half fp8 casts are front-loaded into the first two qb
        # slots so the X8 tail never delays the first scores matmul.
        def z1_hook(qb):
            if qb == 0:
                own_x8(0)
                own_x8(1)
            elif qb == 1:
                own_x8(2)
                own_x8(3)

        z_half(1, post_qb=z1_hook)

        # ---- phase B
        actx.close()
        # V2: hi/lo pairs with ones-columns at dv=1024..1032 in the v8
        # half (vlo half zeroed): the A.V matmul produces the softmax
        # denominator sum(a8)+sum(alo) for free in its third chunk. Key
        # order [own, partner] matches X8. The own half is read straight
        # from local DRAM, never waiting on the collective.
        assert D_ == 1024
        vres = itctx.enter_context(tc.tile_pool(name=f"vres{it}", bufs=1))
        # bv broadcast, applied at the output drain (attention is linear
        # in V and bv is key-constant: out = att(V) + bv exactly)
        bvb = vres.tile([P, D_], F32, name=f"bvb{it}")
        nc.gpsimd.dma_start(bvb, bv_h[None, :].to_broadcast([P, D_]))
        V_sb = vres.tile([P, KC, 2, DV], F8, name=f"V_sb{it}")
        nc.vector.memset(V_sb[:, :, 0, D_:DV], 0.0)
        nc.vector.memset(V_sb[:, :, 1, D_:DV], 1.0)
        nc.sync.dma_start(V_sb[:, 0:KCL, :, :D_], V_loc[:, :, :, :])
        nc.sync.dma_start(
            V_sb[:, KCL:2 * KCL, :, :D_],
            V_gath[bass.ds(partner, 1), :, :, :, :][0],
        )
        alpha = itctx.enter_context(tc.tile_pool(name=f"alpha{it}", bufs=1))
        a16p = itctx.enter_context(tc.tile_pool(name=f"a16p{it}", bufs=3))
        outp = itctx.enter_context(tc.tile_pool(name=f"outp{it}", bufs=2))
        recipp = itctx.enter_context(tc.tile_pool(name=f"recipp{it}", bufs=4))
        psum_s = itctx.enter_context(
            tc.tile_pool(name=f"psum_s{it}", bufs=2, space="PSUM"))
        psum_av = itctx.enter_context(
            tc.tile_pool(name=f"psum_av{it}", bufs=6, space="PSUM"))

        for blk in range(NBLK):
            q0 = blk * qblk
            # A2 pair dim: 0 = a8, 1 = alo (lhsT (a8,alo) x rhs (vlo,v8)
            # = a8.vlo + alo.v8 in one DR instruction)
            A2 = alpha.tile([P, KC, 2, qblk], F8, name=f"A2_{it}")
            for kc in range(KC):
                ps = psum_s.tile([P, qblk], F32, name=f"ps_s{it}")
                k0 = kc * P
                for icp in range(DC // 2):
                    ic = icp * 2
                    nc.tensor.matmul(
                        ps,
                        X8[:, ic:ic + 2, 0, k0:k0 + P],
                        Z8[:, ic:ic + 2, 1, q0:q0 + qblk],
                        start=(ic == 0), stop=False,
                        perf_mode=mybir.MatmulPerfMode.DoubleRow,
                    )
                for ic in range(DC):
                    nc.tensor.matmul(
                        ps,
                        X8[:, ic, :, k0:k0 + P],
                        Z8[:, ic, :, q0:q0 + qblk],
                        start=False, stop=(ic == DC - 1),
                        perf_mode=mybir.MatmulPerfMode.DoubleRow,
                    )
                a16 = a16p.tile([P, qblk], FP16, name=f"a16_{it}",
                                tag="a16", bufs=3)
                nc.scalar.activation(
                    a16, ps, mybir.ActivationFunctionType.Exp,
                    bias=ebias_t, scale=scale,
                )
                nc.vector.tensor_copy(A2[:, kc, 0, :], a16)
                nc.vector.tensor_sub(A2[:, kc, 1, :], a16, A2[:, kc, 0, :])
            # qi-outer: each query-tile's drain overlaps the next tile's
            # matmuls, shrinking the end-of-block (and end-of-kernel) tail
            for qt_l in range(QT_PER_BLK):
                last = (blk == NBLK - 1 and qt_l == QT_PER_BLK - 1)
                avs = [
                    psum_av.tile([P, CH], F32, name=f"av{i}_{it}", tag="av")
                    for i in range(3)
                ]
                qs = qt_l * P
                rc = recipp.tile([P, 1], F32, name=f"rc{it}")
                out_t = outp.tile([P, D_], F32, name=f"out_t{it}")
                row0 = (blk * QT_PER_BLK + qt_l) * P

                def av_matmuls(chs, c0=0, c1=CH):
                    first = True
                    for kcp in range(KC // 2):
                        kc = kcp * 2
                        for ch in chs:
                            nc.tensor.matmul(
                                avs[ch][:, c0:c1],
                                A2[:, kc:kc + 2, 0, qs:qs + P],
                                V_sb[:, kc:kc + 2, 1,
                                     ch * CH + c0:ch * CH + c1],
                                start=first, stop=False,
                                perf_mode=mybir.MatmulPerfMode.DoubleRow,
                            )
                        first = False
                    for kc in range(KC):
                        lastk = (kc == KC - 1)
                        for ch in chs:
                            nc.tensor.matmul(
                                avs[ch][:, c0:c1],
                                A2[:, kc, :, qs:qs + P],
                                V_sb[:, kc, :, ch * CH + c0:ch * CH + c1],
                                start=False, stop=lastk,
                                perf_mode=mybir.MatmulPerfMode.DoubleRow,
                            )

                def drain(ch):
                    # out = av*rc + bv in one DVE op
                    w = CH if ch < 2 else D_ - 2 * CH
                    c0 = ch * CH
                    nc.vector.affine_then_add(
                        out_t[:, c0:c0 + w], avs[ch][:, :w],
                        bvb[:, c0:c0 + w], rc, 0.0)

                if last:
                    # denominator chunk first: its reciprocal + each chunk's
                    # drain + store hide behind the next chunk's matmuls,
                    # shrinking the end-of-kernel tail; the final chunk runs
                    # as two half-width psum regions so even its first
                    # half's drain + store hides behind the second half
                    av_matmuls([2])
                    nc.vector.reciprocal(
                        rc, avs[2][:, D_ - 2 * CH:D_ - 2 * CH + 1])
                    drain(2)
                    nc.sync.dma_start(
                        out_h[row0:row0 + P, 2 * CH:D_],
                        out_t[:, 2 * CH:D_])
                    av_matmuls([0])
                    drain(0)
                    nc.sync.dma_start(
                        out_h[row0:row0 + P, 0:CH], out_t[:, 0:CH])
                    av_matmuls([1])
                    drain(1)
                    nc.sync.dma_start(
                        out_h[row0:row0 + P, CH:2 * CH], out_t[:, CH:2 * CH])
                else:
                    av_matmuls([0, 1, 2])
                    # denominator = column 1024 = chunk 2, local col 336
                    nc.vector.reciprocal(
                        rc, avs[2][:, D_ - 2 * CH:D_ - 2 * CH + 1])
                    for ch in range(3):
                        drain(ch)
                    nc.sync.dma_start(out_h[row0:row0 + P, :], out_t)


_CACHED_NC = None


def make_in_maps(x, Wq, bq, Wk, bk, Wv, bv, sq=None):
    sq = SQ if sq is None else sq
    x = np.asarray(x, dtype=np.float32)
    Wq = np.asarray(Wq, np.float32)
    Wk = np.asarray(Wk, np.float32)
    WvT = np.asarray(Wv, np.float32).T
    in_maps = []
    for c in range(N_CORES):
        b, h = divmod(c, 2)
        # local contraction-dim order = [own j-half, partner j-half]
        if h == 0:
            perm = np.arange(D)
        else:
            perm = np.r_[np.arange(D // 2, D), np.arange(0, D // 2)]
        # full xT: own query-half keys first, then the partner half, both
        # row-permuted into this core's local contraction order
        xb = np.concatenate(
            [x[b][h * sq:(h + 1) * sq], x[b][(1 - h) * sq:(2 - h) * sq]],
            axis=0)
        in_maps.append({
            "xT": np.ascontiguousarray(xb.T[perm]),
            "Wq": np.ascontiguousarray(Wq[:, perm]),
            "WkH": np.ascontiguousarray(Wk[:, h * (D // 2):(h + 1) * (D // 2)]),
            "WvT": np.ascontiguousarray(WvT[perm]),
            "bqT": np.ascontiguousarray(
                np.asarray(bq, np.float32).reshape(D // P, P).T),
            "bv": np.asarray(bv, np.float32),
        })
    return in_maps


def gather_out(results):
    out = np.empty((B, S_FULL, D), np.float32)
    for c in range(N_CORES):
        b, h = divmod(c, 2)
        out[b, h * SQ:(h + 1) * SQ, :] = results[c]["out"]
    return out


def kernel(x, Wq, bq, Wk, bk, Wv, bv):
    from concourse.bass_utils import run_bass_kernel_spmd

    global _CACHED_NC
    if _CACHED_NC is None:
        _CACHED_NC = build_module_cc(S_FULL, SQ, D)
    nc = _CACHED_NC

    in_maps = make_in_maps(x, Wq, bq, Wk, bk, Wv, bv)
    # the device pool occasionally wedges transiently on cold runs
    # (NRT_EXEC_UNIT_UNRECOVERABLE, clears on retry) — retry before failing,
    # resetting the jax/PJRT backend so the retry gets a fresh client
    last_exc = None
    for attempt in range(3):
        try:
            res = run_bass_kernel_spmd(nc, in_maps, list(range(N_CORES)))
            return gather_out(res.results)
        except Exception as e:  # noqa: BLE001 - deliberate broad retry
            last_exc = e
            try:
                import time
                import jax
                time.sleep(2)
                jax.clear_caches()
                jax.extend.backend.clear_backends()
            except Exception:
                pass
    raise last_exc
